# revision 1
# baseline (speedup 1.0000x reference)
"""LIIF-style implicit image upsampler on 8 Trainium2 NeuronCores.

Strategy:
  - Host: 3x3 conv encoder (tiny, 0.04% of FLOPs), per-branch nearest-neighbor
    index + relative-coordinate + ensemble-weight computation from the actual
    `coord` input, and weight packing.  The grading inputs use the canonical
    LIIF cell-center query grid; the gather then has a fixed replicate-4
    structure which the device kernel exploits.  A runtime check verifies the
    structure and falls back to an exact host implementation otherwise.
  - Device (per core = 1/8 of the B*Hq query rows): layer-1 of the MLP
    commutes with the nearest-neighbor gather, so it is computed once over
    the core's ~1152 unique feature pixels; per-query work is the gather
    (identity matmul with a broadcast access pattern), a K=2 matmul for the
    relative coords, 3 hidden layers, ensemble scaling, the output-difference
    layer, and a sigmoid (softmax of 2 == sigmoid of the logit difference).
"""
import numpy as np

import concourse.bacc as bacc
import concourse.mybir as mybir
import concourse.tile as tile
from concourse.bass_utils import run_bass_kernel_spmd

F32 = mybir.dt.float32
F32R = mybir.dt.float32r
AF = mybir.ActivationFunctionType
ALU = mybir.AluOpType

# problem constants (hardcoded per the harness contract)
B, HQ, WQ = 2, 256, 256
HF, WF, C = 64, 64, 256
N_CORES = 8
QROWS_PER_CORE = HQ * B // N_CORES  # 64 query rows of 256 queries
NQ = QROWS_PER_CORE * WQ            # 16384 queries per core
NU = NQ // 512                      # 32 units of 512 queries (2 query rows)
FROWS = 18                          # feature rows shipped per core (16 + 2 halo)
NPIX = FROWS * WF                   # 1152
PADW = WF + 2                       # 66 padded columns
BRANCHES = [(-1, 1), (-1, 1)]  # placeholder, real list below
BRANCHES = [(vx, vy) for vx in (-1, 1) for vy in (-1, 1)]
EPS_SHIFT = 1e-6
CLAMP_EPS = 1e-6

_nc_cache = {}


def _r(ap):
    return ap.bitcast(F32R)


def _build_nc(reps=1, nu=NU, dt_mm=F32R):
    """Build the SPMD single-core program (identical across the 8 cores)."""
    nc = bacc.Bacc(None, target_bir_lowering=False)

    featT_d = nc.dram_tensor("featT", [2, 128, NPIX], dt_mm, kind="ExternalInput")
    xrel_d = nc.dram_tensor("xrel", [4, 2, NQ], dt_mm, kind="ExternalInput")
    xs_d = nc.dram_tensor("xs", [4, NQ], dt_mm, kind="ExternalInput")
    wz1_d = nc.dram_tensor("wz1", [2, 2, 128, 128], dt_mm, kind="ExternalInput")
    wrel_d = nc.dram_tensor("wrel", [2, 2, 128], dt_mm, kind="ExternalInput")
    whid_d = nc.dram_tensor("whid", [3, 2, 2, 128, 128], dt_mm, kind="ExternalInput")
    wd_d = nc.dram_tensor("wd", [2, 128, 1], dt_mm, kind="ExternalInput")
    ident_d = nc.dram_tensor("ident", [128, 128], dt_mm, kind="ExternalInput")
    bias_d = nc.dram_tensor("bias", [128, 9], F32, kind="ExternalInput")
    # dummy input whose shape depends on reps so jit/NEFF caches can't collide
    # across reps variants (the custom-call HLO is otherwise identical)
    dummy_d = nc.dram_tensor("repsig", [1, max(reps, 1)], F32, kind="ExternalInput")
    y_d = nc.dram_tensor("y", [2, NQ], F32, kind="ExternalOutput")
    ysig_d = nc.dram_tensor("ysig", [1, max(reps, 1)], F32, kind="ExternalOutput")

    with tile.TileContext(nc) as tc:
        with (
            tc.tile_pool(name="const", bufs=1) as cpool,
            tc.tile_pool(name="z1pad", bufs=1) as zpool,
            tc.tile_pool(name="io", bufs=3) as iopool,
            tc.tile_pool(name="h", bufs=2) as hpool,
            tc.tile_pool(name="sbc", bufs=2) as sbcpool,
            tc.tile_pool(name="yt", bufs=2) as ypool,
            tc.tile_pool(name="pl1", bufs=3, space="PSUM") as pl1,
            tc.tile_pool(name="pzh", bufs=3, space="PSUM") as pzh,
            tc.tile_pool(name="pdp", bufs=2, space="PSUM") as pdp,
        ):
            def body():
                # ---- resident constants ----
                wz1 = {}
                whid = {}
                wrel = {}
                wd = {}
                for kt in range(2):
                    for ot in range(2):
                        t = cpool.tile([128, 128], dt_mm, tag=f"wz1_{kt}_{ot}")
                        nc.sync.dma_start(t[:], wz1_d[kt, ot])
                        wz1[kt, ot] = t
                for L in range(3):
                    for kt in range(2):
                        for ot in range(2):
                            t = cpool.tile([128, 128], dt_mm, tag=f"wh_{L}_{kt}_{ot}")
                            nc.sync.dma_start(t[:], whid_d[L, kt, ot])
                            whid[L, kt, ot] = t
                for ot in range(2):
                    t = cpool.tile([2, 128], dt_mm, tag=f"wrel_{ot}")
                    nc.sync.dma_start(t[:], wrel_d[ot])
                    wrel[ot] = t
                for kt in range(2):
                    t = cpool.tile([128, 1], dt_mm, tag=f"wd_{kt}")
                    nc.sync.dma_start(t[:], wd_d[kt])
                    wd[kt] = t
                ident = cpool.tile([128, 128], dt_mm, tag="ident")
                nc.sync.dma_start(ident[:], ident_d[:])
                bias = cpool.tile([128, 9], F32, tag="bias")
                nc.sync.dma_start(bias[:], bias_d[:])
                dtile = cpool.tile([1, max(reps, 1)], F32, tag="dummy_sb",
                                   name="dummy_sb")
                nc.sync.dma_start(dtile[:], dummy_d[:])
                nc.sync.dma_start(ysig_d[:], dtile[:])

                # ---- stage A: Z1 over unique pixels, into padded layout ----
                ft = {}
                for kt in range(2):
                    t = cpool.tile([128, NPIX], dt_mm, tag=f"ft_{kt}")
                    nc.sync.dma_start(t[:], featT_d[kt])
                    ft[kt] = t
                z1pad = {}
                for ot in range(2):
                    zt = zpool.tile([128, FROWS, PADW], dt_mm, tag=f"z1pad_{ot}",
                                    name=f"z1pad_{ot}")
                    z1pad[ot] = zt
                ntiles = [(0, 512), (512, 512), (1024, 128)]
                for ot in range(2):
                    zv = z1pad[ot]
                    for (n0, nn) in ntiles:
                        zp = pzh.tile([128, 512], F32, tag="zh")
                        for kt in range(2):
                            nc.tensor.matmul(
                                zp[:, 0:nn], wz1[kt, ot][:], ft[kt][:, n0:n0 + nn],
                                start=(kt == 0), stop=(kt == 1))
                        r0 = n0 // WF
                        nr = nn // WF
                        nc.scalar.activation(
                            zv[:, r0:r0 + nr, 1:1 + WF],
                            zp[:, 0:nn].rearrange("p (a b) -> p a b", a=nr),
                            AF.Copy)
                    # border columns (clamp replication)
                    nc.vector.tensor_copy(zv[:, :, 0:1], zv[:, :, 1:2])
                    nc.vector.tensor_copy(zv[:, :, PADW - 1:PADW], zv[:, :, PADW - 2:PADW - 1])

                # ---- stage B: per-unit MLP ----
                for u in range(nu):
                    q0 = u * 512
                    dp = pdp.tile([1, 512], F32, tag="dp")
                    for br, (vx, vy) in enumerate(BRANCHES):
                        dx = (vx + 1) // 2
                        dw = (vy + 1) // 2
                        relt = iopool.tile([2, 512], dt_mm, tag="relt")
                        nc.sync.dma_start(relt[:], xrel_d[br, :, q0:q0 + 512])
                        st = iopool.tile([1, 512], dt_mm, tag="st")
                        nc.sync.dma_start(st[:], xs_d[br:br + 1, q0:q0 + 512])

                        # L1: gather (identity mm, broadcast AP) + rel mm
                        h1 = hpool.tile([128, 512], dt_mm, tag="h1a")
                        h1b = hpool.tile([128, 512], dt_mm, tag="h1b")
                        h1t = {0: h1, 1: h1b}
                        for ot in range(2):
                            for row in range(2):
                                lr = (2 * u + row + 2) // 4 + dx
                                zl = pl1.tile([128, 260], F32, tag="zl1")
                                mov = z1pad[ot][:, lr, dw:dw + 65].unsqueeze(2)
                                mov = mov.broadcast_to([128, 65, 4])
                                nc.tensor.matmul(zl[:], ident[:], mov,
                                                 start=True, stop=False)
                                nc.tensor.matmul(
                                    zl[:, 2:258], wrel[ot][:],
                                    relt[:, 256 * row:256 * (row + 1)],
                                    start=False, stop=True)
                                nc.scalar.activation(
                                    h1t[ot][:, 256 * row:256 * (row + 1)],
                                    zl[:, 2:258], AF.Relu, bias=bias[:, ot:ot + 1])

                        # hidden layers L2..L4
                        hprev = h1t
                        for L in range(3):
                            hcur = {}
                            for ot in range(2):
                                zh = pzh.tile([128, 512], F32, tag="zh")
                                for kt in range(2):
                                    nc.tensor.matmul(
                                        zh[:], whid[L, kt, ot][:], hprev[kt][:],
                                        start=(kt == 0), stop=(kt == 1))
                                ht = hpool.tile([128, 512], dt_mm, tag=f"h{L}_{ot}")
                                bcol = 2 + 2 * L + ot
                                if (L + ot) % 2 == 0:
                                    nc.scalar.activation(
                                        ht[:], zh[:], AF.Relu,
                                        bias=bias[:, bcol:bcol + 1])
                                else:
                                    nc.vector.tensor_scalar(
                                        ht[:], zh[:], bias[:, bcol:bcol + 1], 0.0,
                                        ALU.add, ALU.max)
                                hcur[ot] = ht
                            hprev = hcur

                        # ensemble scale: broadcast s across partitions, scale h4
                        sbc = sbcpool.tile([128, 512], dt_mm, tag="sbc")
                        nc.gpsimd.partition_broadcast(sbc[:], st[:])
                        for kt in range(2):
                            h4s = hpool.tile([128, 512], dt_mm, tag=f"h4s_{kt}")
                            nc.vector.tensor_tensor(
                                h4s[:], hprev[kt][:], sbc[:], ALU.mult)
                            nc.tensor.matmul(
                                dp[:], wd[kt][:], h4s[:],
                                start=(br == 0 and kt == 0),
                                stop=(br == 3 and kt == 1))

                    # softmax(2) == sigmoid(+/- d)
                    yt = ypool.tile([1, 1024], F32, tag="yt")
                    nc.scalar.activation(yt[:, 0:512], dp[:], AF.Sigmoid,
                                         bias=bias[0:1, 8:9])
                    nc.scalar.activation(yt[:, 512:1024], dp[:], AF.Sigmoid,
                                         scale=-1.0)
                    nc.sync.dma_start(y_d[0:1, q0:q0 + 512], yt[:, 0:512])
                    nc.sync.dma_start(y_d[1:2, q0:q0 + 512], yt[:, 512:1024])

            if reps == 1:
                body()
            else:
                with tc.For_i(0, reps, 1):
                    body()

    nc.compile()
    nc.finalize()
    return nc


def get_nc(reps=1, nu=NU, dt_mm=F32R):
    key = (reps, nu, str(dt_mm))
    if key not in _nc_cache:
        _nc_cache[key] = _build_nc(reps, nu, dt_mm)
    return _nc_cache[key]


# ---------------------------------------------------------------------------
# host-side preparation
# ---------------------------------------------------------------------------

def _conv_feat(inp, conv_w, conv_b):
    """3x3 SAME conv, NCHW/OIHW, via jax on CPU (matches the reference conv)."""
    try:
        import jax
        from jax import lax

        cpu = jax.devices("cpu")[0]

        def f(i, w, b):
            return lax.conv_general_dilated(i, w, (1, 1), "SAME") + b[None, :, None, None]

        with jax.default_device(cpu):
            out = jax.jit(f)(inp, conv_w, conv_b)
        return np.asarray(out)
    except Exception:
        ip = np.pad(inp, ((0, 0), (0, 0), (1, 1), (1, 1)))
        Bn, Ci, H, W = inp.shape
        cols = np.empty((Bn, H, W, Ci, 3, 3), np.float32)
        for kh in range(3):
            for kw in range(3):
                cols[:, :, :, :, kh, kw] = ip[:, :, kh:kh + H, kw:kw + W].transpose(0, 2, 3, 1)
        out = cols.reshape(Bn, H * W, -1) @ conv_w.reshape(conv_w.shape[0], -1).T
        out += conv_b[None, None, :]
        return out.transpose(0, 2, 1).reshape(Bn, conv_w.shape[0], H, W).astype(np.float32)


def _branch_geometry(coord):
    """Per-branch nearest indices and relative coords, exactly as the reference."""
    f32 = np.float32
    rx = f32(1.0) / f32(HF)
    ry = f32(1.0) / f32(WF)
    ihs, iws, rhs, rws = [], [], [], []
    for vx, vy in BRANCHES:
        ch = np.clip(coord[..., 0] + f32(vx) * rx + f32(EPS_SHIFT),
                     f32(-1 + CLAMP_EPS), f32(1 - CLAMP_EPS)).astype(f32)
        cw = np.clip(coord[..., 1] + f32(vy) * ry + f32(EPS_SHIFT),
                     f32(-1 + CLAMP_EPS), f32(1 - CLAMP_EPS)).astype(f32)
        ih = np.clip(np.floor((ch + f32(1.0)) * f32(HF) * f32(0.5)).astype(np.int32), 0, HF - 1)
        iw = np.clip(np.floor((cw + f32(1.0)) * f32(WF) * f32(0.5)).astype(np.int32), 0, WF - 1)
        q_ch = (f32(2.0) * ih.astype(f32) + f32(1.0)) / f32(HF) - f32(1.0)
        q_cw = (f32(2.0) * iw.astype(f32) + f32(1.0)) / f32(WF) - f32(1.0)
        rel_h = ((coord[..., 0] - q_ch) * f32(HF)).astype(f32)
        rel_w = ((coord[..., 1] - q_cw) * f32(WF)).astype(f32)
        ihs.append(ih)
        iws.append(iw)
        rhs.append(rel_h)
        rws.append(rel_w)
    return ihs, iws, rhs, rws


def _grid_ok(ihs, iws):
    """Check the gather indices match the canonical-grid replicate-4 pattern."""
    qi = np.arange(HQ, dtype=np.int64)
    for brn, (vx, vy) in enumerate(BRANCHES):
        dx = (vx + 1) // 2
        dw = (vy + 1) // 2
        ehp = np.clip((qi + 2) // 4 + dx - 1, 0, HF - 1).astype(np.int32)
        ewp = np.clip((qi + 2) // 4 + dw - 1, 0, WF - 1).astype(np.int32)
        if not np.all(ihs[brn] == ehp[None, :, None]):
            return False
        if not np.all(iws[brn] == ewp[None, None, :]):
            return False
    return True


def _host_fallback(inp, coord, cell, conv_w, conv_b, w_in, b_in, w_hid, b_hid,
                   w_out, b_out):
    """Exact reference reimplementation (host, numpy fp32)."""
    feat = _conv_feat(inp, conv_w, conv_b)
    ihs, iws, rhs, rws = _branch_geometry(coord)
    preds, areas = [], []
    for brn in range(4):
        ih, iw = ihs[brn], iws[brn]
        q_feat = np.stack([feat[b][:, ih[b], iw[b]] for b in range(B)])  # [B,C,HQ,WQ]
        rel_h, rel_w = rhs[brn], rws[brn]
        rc_h = np.broadcast_to((cell[:, 0] * HF)[:, None, None], rel_h.shape)
        rc_w = np.broadcast_to((cell[:, 1] * WF)[:, None, None], rel_w.shape)
        x = np.concatenate([
            np.moveaxis(q_feat, 1, -1),
            rel_h[..., None], rel_w[..., None], rc_h[..., None], rc_w[..., None],
        ], axis=-1).astype(np.float32)
        h = np.maximum(x @ w_in + b_in, 0)
        for i in range(w_hid.shape[0]):
            h = np.maximum(h @ w_hid[i] + b_hid[i], 0)
        preds.append(h @ w_out + b_out)
        areas.append(np.abs(rel_h * rel_w) + 1e-9)
    tot = areas[0] + areas[1] + areas[2] + areas[3]
    areas[0], areas[3] = areas[3], areas[0]
    areas[1], areas[2] = areas[2], areas[1]
    ret = sum(p * (a / tot)[..., None] for p, a in zip(preds, areas))
    e = np.exp(ret - ret.max(axis=-1, keepdims=True))
    ret = e / e.sum(axis=-1, keepdims=True)
    return np.moveaxis(ret, -1, 1).astype(np.float32)


def prepare_inputs(inp, coord, cell, conv_w, conv_b, w_in, b_in, w_hid, b_hid,
                   w_out, b_out):
    """Build per-core input maps. Returns (in_maps, grid_ok)."""
    feat = _conv_feat(inp, conv_w, conv_b)          # [B, C, HF, WF]
    ihs, iws, rhs, rws = _branch_geometry(coord)
    if not _grid_ok(ihs, iws):
        return None, False

    # ensemble weights s_b = swapped_area_b / tot
    areas = [np.abs(rhs[b] * rws[b]) + np.float32(1e-9) for b in range(4)]
    tot = areas[0] + areas[1] + areas[2] + areas[3]
    sw = [areas[3] / tot, areas[2] / tot, areas[1] / tot, areas[0] / tot]

    wd = (w_out[:, 0] - w_out[:, 1]).astype(np.float32)        # [256]
    bd = np.float32(b_out[0] - b_out[1])

    wz1 = np.empty((2, 2, 128, 128), np.float32)
    wrel = np.empty((2, 2, 128), np.float32)
    whid = np.empty((3, 2, 2, 128, 128), np.float32)
    for kt in range(2):
        for ot in range(2):
            wz1[kt, ot] = w_in[kt * 128:(kt + 1) * 128, ot * 128:(ot + 1) * 128]
    for ot in range(2):
        wrel[ot] = w_in[256:258, ot * 128:(ot + 1) * 128]
    for L in range(3):
        for kt in range(2):
            for ot in range(2):
                whid[L, kt, ot] = w_hid[L, kt * 128:(kt + 1) * 128,
                                        ot * 128:(ot + 1) * 128]
    wdp = np.empty((2, 128, 1), np.float32)
    wdp[0, :, 0] = wd[:128]
    wdp[1, :, 0] = wd[128:]
    ident = np.eye(128, dtype=np.float32)

    feat_flat = feat.reshape(B, C, HF * WF)

    in_maps = []
    for c in range(N_CORES):
        b = c // 4
        k = c % 4
        # feature rows with clamped halo
        rows = np.clip(np.arange(16 * k - 1, 16 * k + 17), 0, HF - 1)
        fT = feat[b][:, rows, :].reshape(C, NPIX)
        featT = np.ascontiguousarray(fT.reshape(2, 128, NPIX))

        qsl = slice(k * QROWS_PER_CORE, (k + 1) * QROWS_PER_CORE)
        xrel = np.empty((4, 2, NQ), np.float32)
        xs = np.empty((4, NQ), np.float32)
        for brn in range(4):
            xrel[brn, 0] = rhs[brn][b, qsl, :].reshape(NQ)
            xrel[brn, 1] = rws[brn][b, qsl, :].reshape(NQ)
            xs[brn] = sw[brn][b, qsl, :].reshape(NQ)

        # bias pack: cols 0-1 L1(ot0,ot1) with rc folded; 2-7 hidden; col 8 row0=bd
        biasp = np.zeros((128, 9), np.float32)
        rc_h = np.float32(cell[b, 0] * HF)
        rc_w = np.float32(cell[b, 1] * WF)
        b1_eff = (b_in + rc_h * w_in[258] + rc_w * w_in[259]).astype(np.float32)
        biasp[:, 0] = b1_eff[:128]
        biasp[:, 1] = b1_eff[128:]
        for L in range(3):
            biasp[:, 2 + 2 * L] = b_hid[L, :128]
            biasp[:, 3 + 2 * L] = b_hid[L, 128:]
        biasp[0, 8] = bd

        in_maps.append({
            "featT": featT, "xrel": xrel, "xs": xs, "wz1": wz1, "wrel": wrel,
            "whid": whid, "wd": wdp, "ident": ident, "bias": biasp,
        })
    return in_maps, True


def assemble_output(results):
    out = np.empty((B, 2, HQ, WQ), np.float32)
    for c in range(N_CORES):
        b = c // 4
        k = c % 4
        y = results[c]["y"].reshape(2, QROWS_PER_CORE, WQ)
        out[b, :, k * QROWS_PER_CORE:(k + 1) * QROWS_PER_CORE, :] = y
    return out


def kernel(**inputs):
    inputs = {k: np.asarray(v) for k, v in inputs.items()}
    in_maps, ok = prepare_inputs(**inputs)
    if not ok:
        return _host_fallback(**inputs)
    nc = get_nc(reps=1)
    for m in in_maps:
        m["repsig"] = np.zeros((1, 1), np.float32)
    res = run_bass_kernel_spmd(nc, in_maps, core_ids=list(range(N_CORES)))
    return assemble_output(res.results)

